# revision 12
# baseline (speedup 1.0000x reference)
"""Trainium2 Bass kernel for CustomFullyConnectedLayerGoogleTopK.

Math (from the reference, with IN_F == OUT_F == TOTAL_PERM == DIAG_LEN == 4096):
    a_topk = clip(K * softmax(alpha), 0, 1)                    # K = 3687
    Vs     = V * a_topk[:, None]                               # [4096, 4096]
    W[r,c] = Vs[(r - c) % 4096, c]   (scatter has no collisions)
    out    = x @ W.T                                           # [8192, 4096]

Device strategy: data-parallel over batch (8 cores x 1024 rows). The weight
W.T[c, r] = VsT[c, (r - c) % 4096] where VsT = Vs.T. Storing the doubled
array W2 = concat(VsT, VsT, axis=1) [4096, 8192] makes every tile of W.T a
single affine access pattern: element (p, j) of the tile for (k, n) lives at
W2 linear offset (4096 + n*ntile) + p*8191 + k*128*8191 + j.
So the whole matmul streams with plain DMAs - no gather, no transpose.

Modes (GTOPK_MODE env):
  hyb (default): hybrid contraction split - the first KD8*256 contraction
      rows run as fp8 e4m3 perf_mode=DoubleRow matmuls (2 contraction rows
      per PE cell per cycle, i.e. 2x the bf16 issue-rate floor), the rest
      in bf16, accumulating into the same PSUM group. Why: pure-fp8 DR is
      1.93x faster (238 us) but e4m3's 3-bit mantissa gives 0.0266 rms
      noise per operand -> 0.038 rel err, over the 2e-2 gate; correcting
      either operand costs >=3 products per contraction row, which is
      slower than bf16. The hybrid sqrt-scales the noise with the fp8
      fraction. Inputs are pre-scaled by exact powers of 2 (x*32, W*4096)
      so both dtype parts share one 2^-17 unscale, folded into the PSUM
      eviction (tensor_scalar_mul, same DVE cost as the plain copy).
      Dtype phases are block-wise (all DR matmuls of an n-block across the
      8 psum banks, then all bf16, 16 dtype switches total): measured at
      the PE issue-rate floor with zero stalls.
      Measured: KD8=3 -> 419.5 us HW at 2.4 GHz, rel err 0.0183 (KD8=4:
      rel err 0.0198, too close to the gate). NOTE the chip sometimes runs
      the PE at ~2.0 GHz (P0 power state, ntff median MM gap 259 ns vs
      216): the same kernel then measures ~500 us. Not controllable from
      the kernel; per-MM issue floor is the thing to compare.
  dr8: pure fp8 DoubleRow (237.7 us, rel err 0.038 - fails the gate).
  bf16: previous baseline (459.4 us, rel err 1.9e-3).
  fp32r: fp32 storage, reduced-precision multiply (~566 us, 1.2e-4).

Each core: out_slice[1024, 4096] = xT_slice.T @ W.T via PE matmuls,
lhsT = xT tile (stationary), rhs = W.T tile.
"""

import os

import numpy as np
import ml_dtypes

B = 8192  # batch
F = 4096  # in_features == out_features == total_perm == diag_len
NCORES = 8
BS = B // NCORES  # batch rows per core
KTOPK = 3687  # ceil((1 - 0.1) * F * F / F)

X_SCALE = 32.0  # exact power-of-2 prescales for fp8 range use
W_SCALE = 4096.0
OUT_UNSCALE = 1.0 / (X_SCALE * W_SCALE)

_MODE = os.environ.get("GTOPK_MODE", "hyb")
# number of DoubleRow fp8 k-tiles (256 contraction rows each) in hybrid mode;
# the remaining 4096 - 256*KD8 contraction rows run in bf16. Measured absmax
# rel err: KD8=4 -> 0.0198 (too close to the 2e-2 gate), KD8=3 -> 0.0183.
_KD8 = int(os.environ.get("GTOPK_KD8", "3"))

_NC_CACHE = {}
_LAST_RESULTS = None  # stashed BassKernelResults for test harness introspection


def _build_nc_hyb(kd8, f=F, bs=BS):
    """Hybrid kernel: first kd8*256 contraction rows in fp8e4 DoubleRow
    (2 rows/cell/cycle), the rest in bf16, accumulating into the same PSUM
    group. Both inputs are pre-scaled by the same exact powers of 2 so the
    two parts share one unscale factor, folded into the PSUM eviction."""
    import concourse.bass as bass
    import concourse.tile as tile
    from concourse import bacc, mybir

    DR = mybir.MatmulPerfMode.DoubleRow
    dt8 = mybir.dt.float8e4
    dtb = mybir.dt.bfloat16
    n_tile = 512
    kb16 = (f - 256 * kd8) // 128  # bf16 k-tiles
    m_tiles = bs // 128  # 8
    n_tiles = f // n_tile  # 8
    w2w = 2 * f
    c0 = 256 * kd8  # first contraction row handled in bf16

    nc = bacc.Bacc(None, target_bir_lowering=False, debug=False)
    xt8 = nc.dram_tensor("xt8", [c0, bs], dt8, kind="ExternalInput")
    xtb = nc.dram_tensor("xtb", [f - c0, bs], dtb, kind="ExternalInput")
    w28 = nc.dram_tensor("w28", [c0, w2w], dt8, kind="ExternalInput")
    w2b = nc.dram_tensor("w2b", [f - c0, w2w], dtb, kind="ExternalInput")
    out = nc.dram_tensor("out", [bs, f], mybir.dt.float32, kind="ExternalOutput")

    def xt8_src(kd, col0, width):  # [128, 2, width] DR tile kd of x.T slice
        return bass.AP(
            xt8, kd * 256 * bs + col0, [[bs, 128], [128 * bs, 2], [1, width]]
        )

    def xtb_src(kb):  # [128, bs] bf16 tile kb (rows c0 + kb*128 ...)
        return bass.AP(xtb, kb * 128 * bs, [[bs, 128], [1, bs]])

    def w8_src(n, kd, npairs):  # staircase [128, npairs, n_tile] of W.T (fp8)
        return bass.AP(
            w28,
            f + n * n_tile + kd * 256 * (w2w - 1),
            [[w2w - 1, 128], [128 * (w2w - 1), npairs], [1, n_tile]],
        )

    def wb_src(n, kb, nk):  # staircase [128, nk, n_tile] of W.T rows c0+...
        return bass.AP(
            w2b,
            f + n * n_tile - c0 + kb * 128 * (w2w - 1),
            [[w2w - 1, 128], [128 * (w2w - 1), nk], [1, n_tile]],
        )

    with tile.TileContext(nc) as tc:
        with (
            tc.tile_pool(name="xpool", bufs=kd8 + kb16 + 2) as xpool,
            tc.tile_pool(name="wpool", bufs=3 * (kd8 // 2 + kb16 // 2 + 1)) as wpool,
            tc.tile_pool(name="opool", bufs=6) as opool,
            tc.tile_pool(name="ppool", bufs=8, space="PSUM") as ppool,
        ):
            # HAM warmup (see bf16 kernel)
            warm = xpool.tile([128, 128], dtb, name="warm", tag="warm", bufs=1)
            nc.vector.memset(warm[:], 0)
            ps_w = ppool.tile([128, n_tile], mybir.dt.float32, name="ps_w", tag="ps")
            for _ in range(32):
                nc.tensor.matmul(
                    ps_w[:, :128], warm[:], warm[:],
                    start=True, stop=True, skip_group_check=True,
                )

            # n=0 tiles + x cache. First fp8 tile split so the first
            # (x, w) pair lands fast; then alternate x/w DMAs. All input DMAs
            # stay on the sync queue: splitting x onto the gpsimd queue was
            # measured SLOWER (sync-ring throughput dropped ~3x, 8 us stall).
            x8s = []
            xbs = []
            w80 = []
            wb0 = []
            x0a = xpool.tile([128, 2, 256], dt8, name="xt8_0a", tag="xt")
            nc.sync.dma_start(out=x0a[:], in_=xt8_src(0, 0, 256))
            w0 = wpool.tile([128, 2, n_tile], dt8, name="wt8_0_0", tag="wt")
            nc.sync.dma_start(out=w0[:], in_=w8_src(0, 0, 2))
            x0b = xpool.tile([128, 2, bs - 256], dt8, name="xt8_0b", tag="xt")
            nc.sync.dma_start(out=x0b[:], in_=xt8_src(0, 256, bs - 256))
            x8s.append((x0a, x0b))
            w80.append(w0)
            for kd in range(1, kd8):
                xk = xpool.tile([128, 2, bs], dt8, name=f"xt8_{kd}", tag="xt")
                nc.sync.dma_start(out=xk[:], in_=xt8_src(kd, 0, bs))
                wk = wpool.tile([128, 2, n_tile], dt8, name=f"wt8_0_{kd}", tag="wt")
                nc.sync.dma_start(out=wk[:], in_=w8_src(0, kd, 2))
                x8s.append(xk)
                w80.append(wk)
            for kb in range(kb16):
                xk = xpool.tile([128, bs], dtb, name=f"xtb_{kb}", tag="xt")
                nc.sync.dma_start(out=xk[:], in_=xtb_src(kb))
                wk = wpool.tile([128, n_tile], dtb, name=f"wtb_0_{kb}", tag="wt")
                nc.sync.dma_start(out=wk[:], in_=wb_src(0, kb, 1))
                xbs.append(xk)
                wb0.append(wk)

            def x8sl(kd, m):  # fp8 lhsT [128, 2, 128]
                if kd == 0:
                    a, b = x8s[0]
                    if m < 2:
                        return a[:, :, m * 128 : (m + 1) * 128]
                    return b[:, :, (m - 2) * 128 : (m - 1) * 128]
                return x8s[kd][:, :, m * 128 : (m + 1) * 128]

            def xbsl(kb, m):  # bf16 lhsT [128, 128]
                return xbs[kb][:, m * 128 : (m + 1) * 128]

            w8s = w80
            wbs = wb0
            for n in range(n_tiles):
                if n + 1 < n_tiles:
                    nxt8 = []
                    nxtb = []
                    for k2 in range(kd8 // 2):
                        wk = wpool.tile(
                            [128, 4, n_tile], dt8, name=f"wt8_{n+1}_{k2}", tag="wt"
                        )
                        nc.sync.dma_start(out=wk[:], in_=w8_src(n + 1, 2 * k2, 4))
                        nxt8.append(wk)
                    if kd8 % 2:
                        wk = wpool.tile(
                            [128, 2, n_tile], dt8, name=f"wt8_{n+1}_h", tag="wt"
                        )
                        nc.sync.dma_start(
                            out=wk[:], in_=w8_src(n + 1, kd8 - 1, 2)
                        )
                        nxt8.append(wk)
                    for k2 in range(kb16 // 2):
                        wk = wpool.tile(
                            [128, 2, n_tile], dtb, name=f"wtb_{n+1}_{k2}", tag="wt"
                        )
                        nc.sync.dma_start(out=wk[:], in_=wb_src(n + 1, 2 * k2, 2))
                        nxtb.append(wk)
                    if kb16 % 2:
                        wk = wpool.tile(
                            [128, n_tile], dtb, name=f"wtb_{n+1}_h", tag="wt"
                        )
                        nc.sync.dma_start(
                            out=wk[:], in_=wb_src(n + 1, kb16 - 1, 1)
                        )
                        nxtb.append(wk)

                def w8sl(kd):  # fp8 rhs [128, 2, n_tile]
                    if n == 0:
                        return w8s[kd][:]
                    if kd8 % 2 and kd == kd8 - 1:
                        return w8s[kd // 2][:]
                    return w8s[kd // 2][:, 2 * (kd % 2) : 2 * (kd % 2) + 2, :]

                def wbsl(kb):  # bf16 rhs [128, n_tile]
                    if n == 0:
                        return wbs[kb][:]
                    if kb16 % 2 and kb == kb16 - 1:
                        return wbs[kb // 2][:]
                    return wbs[kb // 2][:, kb % 2, :]

                def evict(ps_ap, m, col0, width):
                    o_sb = opool.tile(
                        [128, width], mybir.dt.float32, name="o_sb", tag="o_sb"
                    )
                    nc.vector.tensor_scalar_mul(o_sb[:], ps_ap, OUT_UNSCALE)
                    nc.scalar.dma_start(
                        out=bass.AP(
                            out, m * 128 * f + n * n_tile + col0, [[f, 128], [1, width]]
                        ),
                        in_=o_sb[:],
                    )

                def mm(ps_ap, m, kd_or_kb, is8, first, final):
                    if is8:
                        nc.tensor.matmul(
                            ps_ap, x8sl(kd_or_kb, m), w8sl(kd_or_kb),
                            start=first, stop=final,
                            perf_mode=DR, skip_group_check=True,
                        )
                    else:
                        nc.tensor.matmul(
                            ps_ap, xbsl(kd_or_kb, m), wbsl(kd_or_kb),
                            start=first, stop=final, skip_group_check=True,
                        )

                # Block-wise dtype phases: ALL fp8-DR matmuls of this n-block
                # first (kd-outer / m-inner across the 8 psum banks), then all
                # bf16 — 2 PE weight-dtype switches per block instead of 2 per
                # psum group (a switch costs ~0.6 us of PE stall). Per-m bf16
                # tail staggers group completion so evictions overlap compute.
                kb_half = 3 * kb16 // 4
                last_blk = n == n_tiles - 1
                half = n_tile // 2
                nfull = m_tiles - 1 if last_blk else m_tiles
                pss = [
                    ppool.tile(
                        [128, n_tile], mybir.dt.float32, name=f"ps{m}", tag="ps"
                    )
                    for m in range(nfull)
                ]
                if last_blk:
                    # last group split in column halves so the final eviction
                    # + output DMA moves half as much data after the last MM
                    ph = [
                        ppool.tile(
                            [128, half], mybir.dt.float32, name=f"psh{h}", tag="ps"
                        )
                        for h in range(2)
                    ]
                for kd in range(kd8):
                    for m in range(nfull):
                        mm(pss[m][:], m, kd, True, kd == 0, False)
                    if last_blk:
                        for h in range(2):
                            nc.tensor.matmul(
                                ph[h][:],
                                x8sl(kd, m_tiles - 1),
                                w8sl(kd)[:, :, h * half : (h + 1) * half],
                                start=(kd == 0), stop=False,
                                perf_mode=DR, skip_group_check=True,
                            )
                for kb in range(kb_half):
                    for m in range(nfull):
                        mm(pss[m][:], m, kb, False, False, False)
                    if last_blk:
                        for h in range(2):
                            nc.tensor.matmul(
                                ph[h][:],
                                xbsl(kb, m_tiles - 1),
                                wbsl(kb)[:, h * half : (h + 1) * half],
                                start=False, stop=False,
                                skip_group_check=True,
                            )
                for m in range(nfull):
                    for kb in range(kb_half, kb16):
                        mm(pss[m][:], m, kb, False, False, kb == kb16 - 1)
                    evict(pss[m][:], m, 0, n_tile)
                if last_blk:
                    for h in range(2):
                        for kb in range(kb_half, kb16):
                            nc.tensor.matmul(
                                ph[h][:],
                                xbsl(kb, m_tiles - 1),
                                wbsl(kb)[:, h * half : (h + 1) * half],
                                start=False, stop=(kb == kb16 - 1),
                                skip_group_check=True,
                            )
                        evict(ph[h][:], m_tiles - 1, h * half, half)
                if n + 1 < n_tiles:
                    w8s = nxt8
                    wbs = nxtb
    nc.compile()
    return nc


def _build_nc_dr8(f=F, bs=BS):
    """fp8 e4m3 DoubleRow kernel: 16 DR k-tiles of 256 contraction rows."""
    import concourse.bass as bass
    import concourse.tile as tile
    from concourse import bacc, mybir

    in_dt = mybir.dt.float8e4
    DR = mybir.MatmulPerfMode.DoubleRow
    n_tile = 512
    kd_tiles = f // 256  # 16
    m_tiles = bs // 128  # 8
    n_tiles = f // n_tile  # 8
    w2w = 2 * f

    nc = bacc.Bacc(None, target_bir_lowering=False, debug=False)
    xt = nc.dram_tensor("xt", [f, bs], in_dt, kind="ExternalInput")
    w2 = nc.dram_tensor("w2", [f, w2w], in_dt, kind="ExternalInput")
    out = nc.dram_tensor("out", [bs, f], mybir.dt.float32, kind="ExternalOutput")

    def xt_src(kd, col0, width):  # [128, 2, width] DR tile kd of x.T slice
        return bass.AP(
            xt, kd * 256 * bs + col0, [[bs, 128], [128 * bs, 2], [1, width]]
        )

    def wt_src(n, kd, npairs):  # staircase [128, npairs, n_tile] of W.T
        return bass.AP(
            w2,
            f + n * n_tile + kd * 256 * (w2w - 1),
            [[w2w - 1, 128], [128 * (w2w - 1), npairs], [1, n_tile]],
        )

    with tile.TileContext(nc) as tc:
        with (
            tc.tile_pool(name="xpool", bufs=kd_tiles + 1) as xpool,
            tc.tile_pool(name="wpool", bufs=3 * (kd_tiles // 2)) as wpool,
            tc.tile_pool(name="opool", bufs=6) as opool,
            tc.tile_pool(name="ppool", bufs=8, space="PSUM") as ppool,
        ):
            # HAM warmup: plain (non-DR) fp8 matmuls on scratch fill the
            # PE-idle window between the framework start barrier and first
            # data arrival so real matmuls start at the warm 2.4 GHz clock.
            warm = xpool.tile([128, 128], in_dt, name="warm", tag="warm", bufs=1)
            nc.vector.memset(warm[:], 0)
            ps_w = ppool.tile([128, n_tile], mybir.dt.float32, name="ps_w", tag="ps")
            for _ in range(32):
                nc.tensor.matmul(
                    ps_w[:, :128], warm[:], warm[:],
                    start=True, stop=True, skip_group_check=True,
                )

            # x.T slice cached in SBUF as 16 DR tiles [128, 2, 1024] (pair i
            # = contraction rows kd*256 + i*128 + p). Tile 0 split so the
            # very first (xt, wt) pair is small. Interleave x/w DMAs for n=0.
            xts = []
            wt0 = []
            x0a = xpool.tile([128, 2, 256], in_dt, name="xt0a", tag="xt")
            nc.sync.dma_start(out=x0a[:], in_=xt_src(0, 0, 256))
            w0 = wpool.tile([128, 2, n_tile], in_dt, name="wt0_0", tag="wt")
            nc.sync.dma_start(out=w0[:], in_=wt_src(0, 0, 2))
            x0b = xpool.tile([128, 2, bs - 256], in_dt, name="xt0b", tag="xt")
            nc.sync.dma_start(out=x0b[:], in_=xt_src(0, 256, bs - 256))
            xts.append((x0a, x0b))
            wt0.append(w0)
            for kd in range(1, kd_tiles):
                xk = xpool.tile([128, 2, bs], in_dt, name=f"xt{kd}", tag="xt")
                nc.sync.dma_start(out=xk[:], in_=xt_src(kd, 0, bs))
                wk = wpool.tile([128, 2, n_tile], in_dt, name=f"wt0_{kd}", tag="wt")
                nc.sync.dma_start(out=wk[:], in_=wt_src(0, kd, 2))
                xts.append(xk)
                wt0.append(wk)

            def xsl(kd, m):  # lhsT [128, 2, 128] for (DR k-tile, m-tile)
                if kd == 0:
                    a, b = xts[0]
                    if m < 2:
                        return a[:, :, m * 128 : (m + 1) * 128]
                    return b[:, :, (m - 2) * 128 : (m - 1) * 128]
                return xts[kd][:, :, m * 128 : (m + 1) * 128]

            wts = wt0
            for n in range(n_tiles):
                # prefetch next n's weight tiles (2 DR k-tiles per DMA)
                if n + 1 < n_tiles:
                    nxt = []
                    for k2 in range(kd_tiles // 2):
                        wk = wpool.tile(
                            [128, 4, n_tile], in_dt, name=f"wt{n + 1}_{k2}", tag="wt"
                        )
                        nc.sync.dma_start(
                            out=wk[:], in_=wt_src(n + 1, 2 * k2, 4)
                        )
                        nxt.append(wk)

                def wsl(kd):  # rhs [128, 2, n_tile] for DR k-tile of current n
                    if n == 0:
                        return wts[kd][:]
                    return wts[kd // 2][:, 2 * (kd % 2) : 2 * (kd % 2) + 2, :]

                def evict(ps_ap, m, col0, width):
                    o_sb = opool.tile(
                        [128, width], mybir.dt.float32, name="o_sb", tag="o_sb"
                    )
                    nc.vector.tensor_scalar_mul(o_sb[:], ps_ap, OUT_UNSCALE)
                    nc.scalar.dma_start(
                        out=bass.AP(
                            out, m * 128 * f + n * n_tile + col0, [[f, 128], [1, width]]
                        ),
                        in_=o_sb[:],
                    )

                if n == 0:
                    # Ramp phase: kd-outer / m-inner over the first 3/4 of kd
                    # so each arriving (xt, wt) pair immediately feeds
                    # m_tiles matmuls. Then finish per-m (kd-inner) so the 8
                    # psum banks complete staggered and evictions overlap.
                    kd_half = 3 * kd_tiles // 4
                    pss = [
                        ppool.tile(
                            [128, n_tile], mybir.dt.float32, name=f"ps{m}", tag="ps"
                        )
                        for m in range(m_tiles)
                    ]
                    for kd in range(kd_half):
                        for m in range(m_tiles):
                            nc.tensor.matmul(
                                pss[m][:],
                                xsl(kd, m),
                                wsl(kd),
                                start=(kd == 0),
                                stop=False,
                                perf_mode=DR,
                                skip_group_check=True,
                            )
                    for m in range(m_tiles):
                        for kd in range(kd_half, kd_tiles):
                            nc.tensor.matmul(
                                pss[m][:],
                                xsl(kd, m),
                                wsl(kd),
                                start=False,
                                stop=(kd == kd_tiles - 1),
                                perf_mode=DR,
                                skip_group_check=True,
                            )
                        evict(pss[m][:], m, 0, n_tile)
                else:
                    # m-outer / kd-inner: staggered psum completion overlaps
                    # eviction + output DMA with compute. Final group split
                    # in half column-wise so the last eviction + output DMA
                    # moves half as much data after the last matmul.
                    for m in range(m_tiles):
                        last = n == n_tiles - 1 and m == m_tiles - 1
                        if not last:
                            ps = ppool.tile(
                                [128, n_tile], mybir.dt.float32, name="ps", tag="ps"
                            )
                            for kd in range(kd_tiles):
                                nc.tensor.matmul(
                                    ps[:],
                                    xsl(kd, m),
                                    wsl(kd),
                                    start=(kd == 0),
                                    stop=(kd == kd_tiles - 1),
                                    perf_mode=DR,
                                )
                            evict(ps[:], m, 0, n_tile)
                        else:
                            half = n_tile // 2
                            for h in range(2):
                                ps = ppool.tile(
                                    [128, half], mybir.dt.float32, name="ps", tag="ps"
                                )
                                for kd in range(kd_tiles):
                                    nc.tensor.matmul(
                                        ps[:],
                                        xsl(kd, m),
                                        wsl(kd)[:, :, h * half : (h + 1) * half],
                                        start=(kd == 0),
                                        stop=(kd == kd_tiles - 1),
                                        perf_mode=DR,
                                    )
                                evict(ps[:], m, h * half, half)
                if n + 1 < n_tiles:
                    wts = nxt
    nc.compile()
    return nc


def _build_nc(mode, f=F, bs=BS):
    import concourse.bass as bass
    import concourse.tile as tile
    from concourse import bacc, mybir

    if mode == "bf16":
        in_dt = mybir.dt.bfloat16
        n_tile = 512
    elif mode == "fp32r":
        in_dt = mybir.dt.float32r
        n_tile = 256
    else:
        raise ValueError(mode)

    k_tiles = f // 128
    m_tiles = bs // 128
    n_tiles = f // n_tile
    w2w = 2 * f  # doubled width

    nc = bacc.Bacc(None, target_bir_lowering=False, debug=False)
    xt = nc.dram_tensor("xt", [f, bs], in_dt, kind="ExternalInput")
    w2 = nc.dram_tensor("w2", [f, w2w], in_dt, kind="ExternalInput")
    out = nc.dram_tensor("out", [bs, f], mybir.dt.float32, kind="ExternalOutput")

    def xt_src(k):  # [128, bs] tile k of x.T slice
        return bass.AP(xt, k * 128 * bs, [[bs, 128], [1, bs]])

    def wt_src(n, k):  # staircase [128, n_tile] tile of W.T
        return bass.AP(
            w2, f + n * n_tile + k * 128 * (w2w - 1), [[w2w - 1, 128], [1, n_tile]]
        )

    with tile.TileContext(nc) as tc:
        with (
            tc.tile_pool(name="xpool", bufs=k_tiles + 1) as xpool,
            tc.tile_pool(name="wpool", bufs=3 * (k_tiles // 2)) as wpool,
            tc.tile_pool(name="opool", bufs=6) as opool,
            tc.tile_pool(name="ppool", bufs=8, space="PSUM") as ppool,
        ):
            # HAM warmup: N=128 matmuls on scratch fill the PE-idle window
            # between the framework start barrier and first data arrival, so
            # real matmuls start at the warm 2.4 GHz clock.
            warm = xpool.tile([128, 128], in_dt, name="warm", tag="warm", bufs=1)
            nc.vector.memset(warm[:], 0)
            ps_w = ppool.tile([128, n_tile], mybir.dt.float32, name="ps_w", tag="ps")
            for _ in range(32):
                nc.tensor.matmul(
                    ps_w[:, :128], warm[:], warm[:],
                    start=True, stop=True, skip_group_check=True,
                )

            # x.T slice cached in SBUF as separate tiles so the scheduler can
            # start matmuls as soon as individual tiles land. xt[0] is split
            # so the very first (xt, wt) pair is only 192 KB. Interleave x/w
            # DMAs for n=0 so pairs arrive together.
            xts = []
            wt0 = []
            split0 = bs > 256
            if split0:
                x0a = xpool.tile([128, 256], in_dt, name="xt0a", tag="xt")
                nc.sync.dma_start(
                    out=x0a[:], in_=bass.AP(xt, 0, [[bs, 128], [1, 256]])
                )
                w0 = wpool.tile([128, n_tile], in_dt, name="wt0_0", tag="wt")
                nc.sync.dma_start(out=w0[:], in_=wt_src(0, 0))
                x0b = xpool.tile([128, bs - 256], in_dt, name="xt0b", tag="xt")
                nc.sync.dma_start(
                    out=x0b[:], in_=bass.AP(xt, 256, [[bs, 128], [1, bs - 256]])
                )
                xts.append((x0a, x0b))
                wt0.append(w0)
            else:
                x0 = xpool.tile([128, bs], in_dt, name="xt0", tag="xt")
                nc.sync.dma_start(out=x0[:], in_=xt_src(0))
                w0 = wpool.tile([128, n_tile], in_dt, name="wt0_0", tag="wt")
                nc.sync.dma_start(out=w0[:], in_=wt_src(0, 0))
                xts.append(x0)
                wt0.append(w0)
            for k in range(1, k_tiles):
                xk = xpool.tile([128, bs], in_dt, name=f"xt{k}", tag="xt")
                nc.sync.dma_start(out=xk[:], in_=xt_src(k))
                wk = wpool.tile([128, n_tile], in_dt, name=f"wt0_{k}", tag="wt")
                nc.sync.dma_start(out=wk[:], in_=wt_src(0, k))
                xts.append(xk)
                wt0.append(wk)

            def xsl(k, m):  # lhsT block [128, 128] for (k-tile, m-tile)
                if k == 0 and split0:
                    a, b = xts[0]
                    if m < 2:
                        return a[:, m * 128 : (m + 1) * 128]
                    return b[:, (m - 2) * 128 : (m - 1) * 128]
                return xts[k][:, m * 128 : (m + 1) * 128]

            wts = wt0
            for n in range(n_tiles):
                # prefetch next n's weight tiles (2 k-tiles per DMA: halves
                # the ~0.6us-per-DMA issue load on the sync sequencer)
                if n + 1 < n_tiles:
                    nxt = []
                    for k2 in range(k_tiles // 2):
                        wk = wpool.tile(
                            [128, 2, n_tile], in_dt, name=f"wt{n + 1}_{k2}", tag="wt"
                        )
                        nc.sync.dma_start(
                            out=wk[:],
                            in_=bass.AP(
                                w2,
                                f + (n + 1) * n_tile + k2 * 256 * (w2w - 1),
                                [[w2w - 1, 128], [128 * (w2w - 1), 2], [1, n_tile]],
                            ),
                        )
                        nxt.append(wk)
                def wsl(k):  # rhs [128, n_tile] for k-tile of current n
                    if n == 0:
                        return wts[k][:]
                    return wts[k // 2][:, k % 2, :]

                def evict(ps_ap, m, col0, width):
                    o_sb = opool.tile(
                        [128, width], mybir.dt.float32, name="o_sb", tag="o_sb"
                    )
                    nc.vector.tensor_copy(o_sb[:], ps_ap)
                    nc.scalar.dma_start(
                        out=bass.AP(
                            out, m * 128 * f + n * n_tile + col0, [[f, 128], [1, width]]
                        ),
                        in_=o_sb[:],
                    )

                if n == 0:
                    # Ramp phase: k-outer / m-inner over the first half of k
                    # so each arriving (xt[k], wt[k]) pair immediately feeds
                    # m_tiles matmuls (PE starts as soon as the first pair
                    # lands). Then finish per-m (k-inner) so the 8 psum banks
                    # complete staggered and evictions overlap compute.
                    k_half = min(3 * k_tiles // 4, k_tiles)
                    pss = [
                        ppool.tile([128, n_tile], mybir.dt.float32, name=f"ps{m}", tag="ps")
                        for m in range(m_tiles)
                    ]
                    for k in range(k_half):
                        for m in range(m_tiles):
                            nc.tensor.matmul(
                                pss[m][:],
                                xsl(k, m),
                                wsl(k),
                                start=(k == 0),
                                stop=False,
                                skip_group_check=True,
                            )
                    for m in range(m_tiles):
                        for k in range(k_half, k_tiles):
                            nc.tensor.matmul(
                                pss[m][:],
                                xsl(k, m),
                                wsl(k),
                                start=False,
                                stop=(k == k_tiles - 1),
                                skip_group_check=True,
                            )
                        evict(pss[m][:], m, 0, n_tile)
                else:
                    # m-outer / k-inner: staggered psum completion overlaps
                    # eviction + output DMA with compute. The very last group
                    # is split in half column-wise so the final eviction +
                    # output DMA (whose ~2us HBM write receipt is on the
                    # critical path) moves half as much data after the last
                    # matmul.
                    for m in range(m_tiles):
                        last = n == n_tiles - 1 and m == m_tiles - 1
                        if not last:
                            ps = ppool.tile(
                                [128, n_tile], mybir.dt.float32, name="ps", tag="ps"
                            )
                            for k in range(k_tiles):
                                nc.tensor.matmul(
                                    ps[:],
                                    xsl(k, m),
                                    wsl(k),
                                    start=(k == 0),
                                    stop=(k == k_tiles - 1),
                                )
                            evict(ps[:], m, 0, n_tile)
                        else:
                            half = n_tile // 2
                            for h in range(2):
                                ps = ppool.tile(
                                    [128, half], mybir.dt.float32, name="ps", tag="ps"
                                )
                                for k in range(k_tiles):
                                    nc.tensor.matmul(
                                        ps[:],
                                        xsl(k, m),
                                        wsl(k)[:, h * half : (h + 1) * half],
                                        start=(k == 0),
                                        stop=(k == k_tiles - 1),
                                    )
                                evict(ps[:], m, h * half, half)
                if n + 1 < n_tiles:
                    wts = nxt
    nc.compile()
    return nc


def _get_nc(mode):
    if mode not in _NC_CACHE:
        if mode == "dr8":
            _NC_CACHE[mode] = _build_nc_dr8()
        elif mode == "hyb":
            _NC_CACHE[mode] = _build_nc_hyb(_KD8)
        else:
            _NC_CACHE[mode] = _build_nc(mode)
    return _NC_CACHE[mode]


def _soft_topk_scale(alpha):
    a = alpha.astype(np.float64)
    e = np.exp(a - a.max())
    return np.clip(KTOPK * (e / e.sum()), 0.0, 1.0).astype(np.float32)


def kernel(x, V, alpha):
    global _LAST_RESULTS
    from concourse.bass_utils import run_bass_kernel_spmd

    x = np.asarray(x, dtype=np.float32)
    V = np.asarray(V, dtype=np.float32)
    alpha = np.asarray(alpha, dtype=np.float32)

    a_topk = _soft_topk_scale(alpha)
    VsT = np.ascontiguousarray((V * a_topk[:, None]).T)  # [c, p]
    W2 = np.concatenate([VsT, VsT], axis=1)  # [F, 2F]
    xT = np.ascontiguousarray(x.T)  # [F, B]

    mode = _MODE
    if mode == "bf16":
        W2 = W2.astype(ml_dtypes.bfloat16)
        xT = xT.astype(ml_dtypes.bfloat16)
    elif mode == "dr8":
        W2 = (W2 * np.float32(W_SCALE)).astype(ml_dtypes.float8_e4m3)
        xT = (xT * np.float32(X_SCALE)).astype(ml_dtypes.float8_e4m3)
    elif mode == "hyb":
        c0 = 256 * _KD8
        W2 *= np.float32(W_SCALE)  # exact power-of-2 scales
        xT *= np.float32(X_SCALE)
        W28 = W2[:c0].astype(ml_dtypes.float8_e4m3)
        W2B = W2[c0:].astype(ml_dtypes.bfloat16)
        xT8 = xT[:c0].astype(ml_dtypes.float8_e4m3)
        xTB = xT[c0:].astype(ml_dtypes.bfloat16)

    nc = _get_nc(mode)
    if mode == "hyb":
        in_maps = [
            {
                "xt8": np.ascontiguousarray(xT8[:, i * BS : (i + 1) * BS]),
                "xtb": np.ascontiguousarray(xTB[:, i * BS : (i + 1) * BS]),
                "w28": W28,
                "w2b": W2B,
            }
            for i in range(NCORES)
        ]
    else:
        in_maps = [
            {"xt": np.ascontiguousarray(xT[:, i * BS : (i + 1) * BS]), "w2": W2}
            for i in range(NCORES)
        ]
    kwargs = {}
    if os.environ.get("GTOPK_TRACE"):
        try:
            import antenv.axon_hooks  # noqa: F401  (trace needs the hook)

            kwargs["trace"] = True
        except ImportError:
            pass
    res = run_bass_kernel_spmd(nc, in_maps, core_ids=list(range(NCORES)), **kwargs)
    _LAST_RESULTS = res
    return np.concatenate([r["out"] for r in res.results], axis=0)
